# revision 22
# baseline (speedup 1.0000x reference)
"""Trainium2 Bass kernel for nn_Attention: single-head attention,
B=32, N=1024, DIM=512, fp32.

    q = X @ Wq.T ; k = X @ Wk.T ; v = X @ Wv.T
    out = softmax((q k^T)/sqrt(D)) @ v

Strategy (8 NeuronCores, data-parallel over batch, 4 batches/core):
  - Host folds A = (Wq.T @ Wk)/sqrt(D)  so scores = X A X.T  — saves one
    projection-sized matmul per batch and needs only X (transposed) on
    device.
  - All tensors live transposed on device: XT [d, n], GT = (X A).T,
    V [n, e], scores ST [k, q] (k on partitions).  Softmax runs along
    the partition axis: exp on ScalarE, partition sums via a ones-vector
    matmul, broadcast of 1/denom via a rank-1 ones matmul, normalization
    fused into the PSUM->SBUF eviction on VectorE.  Attention output is
    produced transposed (OT [e, q]) and the host transposes it back.
  - Matmuls use the float32r PE mode (full-rate fp32 streaming).
"""
import numpy as np

B, N, D = 32, 1024, 512
NCORES = 8
BPC = B // NCORES          # batches per core
DC = D // 128              # 4 chunks of 128 along d / e
KC = N // 128              # 8 chunks of 128 along k
QH = N // 512              # 2 q-halves of 512

_cache = {}


def _split_sync_waits(nc):
    """walrus on this image accepts at most ONE semaphore wait per
    instruction; hoist extras onto InstNoOp carriers on the same engine
    (same-engine program order preserves the gating)."""
    import concourse.mybir as mybir

    ctr = 0
    for f in nc.m.functions:
        for bb in f.blocks:
            out = []
            changed = False
            for ins in bb.instructions:
                si = getattr(ins, "sync_info", None)
                waits = list(si.on_wait) if si and si.on_wait else []
                if len(waits) > 1:
                    for w in waits[:-1]:
                        ctr += 1
                        out.append(
                            mybir.InstNoOp(
                                name=f"wsplit-{ctr}",
                                engine=ins.engine,
                                bass_nofuse=True,
                                sync_info=mybir.SyncInfo(on_wait=[w], on_update=[]),
                            )
                        )
                    ins.sync_info = mybir.SyncInfo(
                        on_wait=waits[-1:], on_update=list(si.on_update or [])
                    )
                    changed = True
                out.append(ins)
            if changed:
                bb.instructions[:] = out


def _build():
    import concourse.bass as bass
    import concourse.mybir as mybir
    import concourse.tile as tile

    f32 = mybir.dt.float32
    f32r = mybir.dt.float32r
    Exp = mybir.ActivationFunctionType.Exp

    nc = bass.Bass(target_bir_lowering=False)

    xt = nc.dram_tensor("xt", [BPC, D, N], f32, kind="ExternalInput")
    a_mat = nc.dram_tensor("a_mat", [D, D], f32, kind="ExternalInput")
    wvt = nc.dram_tensor("wvt", [D, D], f32, kind="ExternalInput")
    ones_col_d = nc.dram_tensor("ones_col", [128, 1], f32, kind="ExternalInput")
    ones_row_d = nc.dram_tensor("ones_row", [1, 128], f32, kind="ExternalInput")
    out_t = nc.dram_tensor("out_t", [BPC, D, N], f32, kind="ExternalOutput")

    with tile.TileContext(nc) as tc:
        with (
            tc.tile_pool(name="wpool", bufs=1) as wpool,
            tc.tile_pool(name="xpool", bufs=2) as xpool,
            tc.tile_pool(name="gpool", bufs=2) as gpool,
            tc.tile_pool(name="vpool", bufs=2) as vpool,
            tc.tile_pool(name="epool", bufs=3) as epool,
            tc.tile_pool(name="eapool", bufs=2) as eapool,
            tc.tile_pool(name="opool", bufs=2) as opool,
            tc.tile_pool(name="rpool", bufs=2) as rpool,
            tc.tile_pool(name="dpool", bufs=2) as dpool,
            tc.tile_pool(name="ps_ws", bufs=3, space="PSUM") as ps_ws,
            tc.tile_pool(name="ps_ot", bufs=4, space="PSUM") as ps_ot,
            tc.tile_pool(name="ps_den", bufs=1, space="PSUM") as ps_den,
        ):
            # --- weights / constants (once) ---
            a_sb = []
            for c in range(DC):
                t = wpool.tile([128, D], f32, tag=f"a{c}", name=f"a_sb{c}")
                nc.sync.dma_start(
                    t[:].bitcast(f32r),
                    a_mat[c * 128:(c + 1) * 128, :].bitcast(f32r),
                )
                a_sb.append(t)
            ones_col = wpool.tile([128, 1], f32, tag="onec")
            nc.sync.dma_start(ones_col[:].bitcast(f32r), ones_col_d[:].bitcast(f32r))
            ones_row = wpool.tile([1, 128], f32, tag="oner")
            nc.sync.dma_start(ones_row[:].bitcast(f32r), ones_row_d[:].bitcast(f32r))
            wvt_sb = []
            for c in range(DC):
                t = wpool.tile([128, D], f32, tag=f"wvt{c}", name=f"wvt_sb{c}")
                nc.sync.dma_start(
                    t[:].bitcast(f32r),
                    wvt[c * 128:(c + 1) * 128, :].bitcast(f32r),
                )
                wvt_sb.append(t)

            def load_xt(b):
                xts = []
                for c in range(DC):
                    t = xpool.tile([128, N], f32, tag=f"xt{c}", name=f"xt_b{b}c{c}")
                    nc.scalar.dma_start(
                        t[:].bitcast(f32r),
                        xt[b, c * 128:(c + 1) * 128, :].bitcast(f32r),
                    )
                    xts.append(t)
                return xts

            def gt_phase(b, xts):
                gt_sb = gpool.tile([128, DC * N], f32, tag="gt", name=f"gt_b{b}")
                for m in range(DC):
                    for h in range(QH):
                        pg = ps_ws.tile([128, 512], f32, tag="ws", name=f"pg{b}{m}{h}")
                        for k4 in range(DC):
                            nc.tensor.matmul(
                                pg[:],
                                a_sb[k4][:, m * 128:(m + 1) * 128].bitcast(f32r),
                                xts[k4][:, h * 512:(h + 1) * 512].bitcast(f32r),
                                start=(k4 == 0), stop=(k4 == DC - 1),
                            )
                        nc.scalar.copy(
                            gt_sb[:, m * N + h * 512:m * N + (h + 1) * 512].bitcast(f32r),
                            pg[:],
                        )
                return gt_sb

            def v_phase(b, xts):
                v_sb = vpool.tile([128, KC * D], f32, tag="v", name=f"v_b{b}")
                for m in range(KC):
                    pv = ps_ws.tile([128, 512], f32, tag="ws", name=f"pv{b}{m}")
                    for k4 in range(DC):
                        nc.tensor.matmul(
                            pv[:],
                            xts[k4][:, m * 128:(m + 1) * 128].bitcast(f32r),
                            wvt_sb[k4][:].bitcast(f32r),
                            start=(k4 == 0), stop=(k4 == DC - 1),
                        )
                    nc.scalar.copy(
                        v_sb[:, m * D:(m + 1) * D].bitcast(f32r), pv[:]
                    )
                return v_sb

            def gt_phase_k4outer(b, xts):
                gt_sb = gpool.tile([128, DC * N], f32, tag="gt", name=f"gt_b{b}")
                grp = {}
                for i, (m, h) in enumerate([(m, h) for m in range(DC) for h in range(QH)]):
                    pool = [ps_ws, ps_ws, ps_ws, ps_ot, ps_ot, ps_ot, ps_ot, ps_den][i]
                    grp[(m, h)] = pool.tile([128, 512], f32, tag=["ws", "ws", "ws", "ot", "ot", "ot", "ot", "den"][i], name=f"pg0_{m}{h}")
                for k4 in range(DC):
                    for m in range(DC):
                        for h in range(QH):
                            nc.tensor.matmul(
                                grp[(m, h)][:],
                                a_sb[k4][:, m * 128:(m + 1) * 128].bitcast(f32r),
                                xts[k4][:, h * 512:(h + 1) * 512].bitcast(f32r),
                                start=(k4 == 0), stop=(k4 == DC - 1),
                            )
                for m in range(DC):
                    for h in range(QH):
                        nc.scalar.copy(
                            gt_sb[:, m * N + h * 512:m * N + (h + 1) * 512].bitcast(f32r),
                            grp[(m, h)][:],
                        )
                return gt_sb

            def v_phase_k4outer(b, xts):
                v_sb = vpool.tile([128, KC * D], f32, tag="v", name=f"v_b{b}")
                grp = {}
                for m in range(KC):
                    pool = [ps_ws, ps_ws, ps_ws, ps_ot, ps_ot, ps_ot, ps_ot, ps_den][m]
                    grp[m] = pool.tile([128, 512], f32, tag=["ws", "ws", "ws", "ot", "ot", "ot", "ot", "den"][m], name=f"pv0_{m}")
                for k4 in range(DC):
                    for m in range(KC):
                        nc.tensor.matmul(
                            grp[m][:],
                            xts[k4][:, m * 128:(m + 1) * 128].bitcast(f32r),
                            wvt_sb[k4][:].bitcast(f32r),
                            start=(k4 == 0), stop=(k4 == DC - 1),
                        )
                for m in range(KC):
                    nc.scalar.copy(
                        v_sb[:, m * D:(m + 1) * D].bitcast(f32r), grp[m][:]
                    )
                return v_sb

            def ph2_compute(b, h, xts, gt_sb, v_sb):
                p_den = ps_den.tile([1, 512], f32, tag="den", name=f"den{b}{h}")
                p_ot = [ps_ot.tile([128, 512], f32, tag="ot", name=f"p_ot{b}{h}{m}")
                        for m in range(DC)]
                # E running sum on VectorE (replaces 7 of 8 ones-matmuls on PE)
                ea = [eapool.tile([128, 512], f32, tag="ea0", name=f"ea0_{b}{h}"),
                      eapool.tile([128, 512], f32, tag="ea1", name=f"ea1_{b}{h}")]
                for kc in range(KC):
                    p_st = ps_ws.tile([128, 512], f32, tag="ws", name=f"st{b}{h}{kc}")
                    for k4 in range(DC):
                        nc.tensor.matmul(
                            p_st[:],
                            xts[k4][:, kc * 128:(kc + 1) * 128].bitcast(f32r),
                            gt_sb[:, k4 * N + h * 512:k4 * N + (h + 1) * 512].bitcast(f32r),
                            start=(k4 == 0), stop=(k4 == DC - 1),
                        )
                    e_sb = epool.tile([128, 512], f32, tag="e", name=f"e{b}{h}{kc}")
                    nc.scalar.activation(e_sb[:].bitcast(f32r), p_st[:], Exp)
                    if kc == 0:
                        nc.vector.tensor_copy(ea[0][:], e_sb[:])
                    elif kc < KC - 1:
                        nc.vector.tensor_add(
                            ea[kc % 2][:], ea[(kc + 1) % 2][:], e_sb[:]
                        )
                    for m in range(DC):
                        nc.tensor.matmul(
                            p_ot[m][:],
                            v_sb[:, kc * D + m * 128:kc * D + (m + 1) * 128].bitcast(f32r),
                            e_sb[:].bitcast(f32r),
                            start=(kc == 0), stop=(kc == KC - 1),
                        )
                ea_r = eapool.tile([128, 512], f32, tag="ear", name=f"ear{b}{h}")
                nc.vector.tensor_add(
                    ea_r[:].bitcast(f32r), ea[(KC - 2) % 2][:],
                    e_sb[:]
                )
                nc.tensor.matmul(
                    p_den[:], ones_col[:].bitcast(f32r), ea_r[:].bitcast(f32r),
                    start=True, stop=True,
                )
                return p_den, p_ot

            def ph2_evict(b, h, p_den, p_ot):
                den_sb = dpool.tile([1, 512], f32, tag="densb", name=f"dsb{b}{h}")
                nc.vector.tensor_copy(den_sb[:].bitcast(f32r), p_den[:])
                otraw = opool.tile([128, DC * 512], f32, tag="otraw", name=f"orw{b}{h}")
                for m in range(DC):
                    nc.scalar.copy(otraw[:, m * 512:(m + 1) * 512], p_ot[m][:])
                return den_sb, otraw

            def ph2_norm(b, h, den_sb, otraw):
                p_bc = ps_ws.tile([128, 512], f32, tag="ws", name=f"bc{b}{h}")
                nc.tensor.matmul(
                    p_bc[:], ones_row[:].bitcast(f32r), den_sb[:].bitcast(f32r)
                )
                ln_sb = rpool.tile([128, 512], f32, tag="ln", name=f"ln{b}{h}")
                nc.scalar.activation(ln_sb[:], p_bc[:], mybir.ActivationFunctionType.Ln)
                rc_sb = rpool.tile([128, 512], f32, tag="rc", name=f"rc{b}{h}")
                nc.scalar.activation(rc_sb[:], ln_sb[:], mybir.ActivationFunctionType.Exp,
                                     scale=-1.0)
                ot_sb = opool.tile([128, DC * 512], f32, tag="ot", name=f"osb{b}{h}")
                for g in range(2):
                    for m in (2 * g, 2 * g + 1):
                        nc.vector.tensor_mul(
                            ot_sb[:, m * 512:(m + 1) * 512],
                            otraw[:, m * 512:(m + 1) * 512], rc_sb[:]
                        )
                    (nc.scalar if g == 0 else nc.sync).dma_start(
                        out_t[b, g * 256:(g + 1) * 256, h * 512:(h + 1) * 512].rearrange(
                            "(m p) q -> p m q", p=128
                        ),
                        ot_sb[:, g * 1024:(g + 1) * 1024].rearrange(
                            "p (m q) -> p m q", m=2
                        ),
                    )

            # software pipeline: phase-1 of batch b+1 fills the PE boundary
            # stalls of batch b's phase-2 (PE executes in program order).
            xts = load_xt(0)
            gt_sb = gt_phase_k4outer(0, xts)
            v_sb = v_phase_k4outer(0, xts)
            state = (xts, gt_sb, v_sb)
            for b in range(BPC):
                xts, gt_sb, v_sb = state
                p_den, p_ot = ph2_compute(b, 0, xts, gt_sb, v_sb)
                den_sb, otraw = ph2_evict(b, 0, p_den, p_ot)
                if b + 1 < BPC:
                    nxts = load_xt(b + 1)
                    ngt = gt_phase(b + 1, nxts)
                ph2_norm(b, 0, den_sb, otraw)
                p_den, p_ot = ph2_compute(b, 1, xts, gt_sb, v_sb)
                den_sb, otraw = ph2_evict(b, 1, p_den, p_ot)
                if b + 1 < BPC:
                    nv = v_phase(b + 1, nxts)
                    state = (nxts, ngt, nv)
                ph2_norm(b, 1, den_sb, otraw)
    return nc


def _prepare_inputs(embeddings, Wq, Wk, Wv):
    xt_all = np.ascontiguousarray(embeddings.transpose(0, 2, 1)).astype(
        np.float32, copy=False
    )
    a_mat = (
        Wq.astype(np.float64).T @ Wk.astype(np.float64) / np.sqrt(float(D))
    ).astype(np.float32)
    wvt = np.ascontiguousarray(Wv.T).astype(np.float32, copy=False)
    ones_col = np.ones((128, 1), np.float32)
    ones_row = np.ones((1, 128), np.float32)
    in_maps = []
    for i in range(NCORES):
        in_maps.append(
            {
                "xt": np.ascontiguousarray(xt_all[i * BPC:(i + 1) * BPC]),
                "a_mat": a_mat,
                "wvt": wvt,
                "ones_col": ones_col,
                "ones_row": ones_row,
            }
        )
    return in_maps


def _get_nc():
    if "nc" not in _cache:
        nc = _build()
        _split_sync_waits(nc)
        _cache["nc"] = nc
    return _cache["nc"]


def _assemble(results):
    out = np.empty((B, N, D), np.float32)
    for i in range(NCORES):
        ot = results[i]["out_t"]  # [BPC, D, N]
        out[i * BPC:(i + 1) * BPC] = ot.transpose(0, 2, 1)
    return out


def kernel(embeddings, Wq, Wk, Wv):
    from concourse.bass_utils import run_bass_kernel_spmd

    embeddings = np.asarray(embeddings, dtype=np.float32)
    in_maps = _prepare_inputs(
        embeddings, np.asarray(Wq), np.asarray(Wk), np.asarray(Wv)
    )
    res = run_bass_kernel_spmd(_get_nc(), in_maps, list(range(NCORES)))
    return _assemble(res.results)
